# revision 1
# baseline (speedup 1.0000x reference)
"""GATv2 block kernel for 8 Trainium2 NeuronCores (Bass/Tile).

Strategy (graph/data parallel over destination nodes):
  - Host sorts edges by destination, shards destination nodes across the
    8 cores (6250 nodes each, padded to 6272 = 49 tiles of 128).
  - Per destination-node tile, edges are padded to multiples of 128
    ("chunks"); chunk counts per tile are maxed across cores so one SPMD
    program serves all 8 cores.
  - Host supplies x[src] pre-gathered AND transposed (x_srcT) so the
    device computes per-edge xl[src] = w_l @ x_src via matmuls with a
    constant stationary operand (no indirect DMA gathers).
  - Segment softmax + scatter-add are matmuls against indicator matrices
    I[e,n] = (dst_local[e] == n) built on-device with is_equal.
  - exp/leaky_relu live in one ACT table set; silu + sqrt run in a tail
    phase (one table switch each).
"""

import numpy as np
import ml_dtypes

BF16 = ml_dtypes.bfloat16

P = 128
HEADS = 4
HEAD_DIM = 32
OUT_DIM = 128
IN_DIM = 128
EDGE_DIM = 10
NEG_SLOPE = 0.2
LN_EPS = 1e-5
N_CORES = 8
SUPER = 4  # chunks per superchunk (free dim 512)

_CACHE = {}


_PATCHED = []


def _enable_ldw_opt():
    # walrus LDWEIGHTS double-buffering: lets weight loads overlap in-flight
    # matmuls instead of serializing every LDW+MM pair.
    if _PATCHED:
        return
    from concourse import bass_utils as bu
    orig = bu.run_command

    def run_command(argv, **kwargs):
        argv = ['--enable-ldw-opt=true' if a == '--enable-ldw-opt=false' else a
                for a in argv]
        return orig(argv, **kwargs)

    bu.run_command = run_command
    _PATCHED.append(True)


def _build_program(C_list, trivial_affine):
    import concourse.bacc as bacc
    import concourse.bass as bass
    import concourse.tile as tile
    from concourse import mybir

    f32 = mybir.dt.float32
    bf16 = mybir.dt.bfloat16
    AT = mybir.ActivationFunctionType
    OP = mybir.AluOpType

    NT = len(C_list)                       # 49 node tiles per core
    CMAX = max(C_list)
    TOTAL_CHUNKS = sum(C_list)
    NPC_PAD = NT * P                       # 6272
    EW = TOTAL_CHUNKS * P                  # padded edges per core

    nc = bacc.Bacc('TRN2', target_bir_lowering=False, debug=False,
                   enable_asserts=True, num_devices=N_CORES)

    # ---- external inputs ----
    x_srcT = nc.dram_tensor('x_srcT', [P, EW], bf16, kind='ExternalInput')
    attrT = nc.dram_tensor('attrT', [EDGE_DIM, EW], bf16, kind='ExternalInput')
    dstrow = nc.dram_tensor('dstrow', [1, EW], bf16, kind='ExternalInput')
    dstloc = nc.dram_tensor('dstloc', [P, TOTAL_CHUNKS], f32, kind='ExternalInput')
    x_ownT = nc.dram_tensor('x_ownT', [P, NPC_PAD], bf16, kind='ExternalInput')
    x_own = nc.dram_tensor('x_own', [NPC_PAD, P], f32, kind='ExternalInput')
    w_lT = nc.dram_tensor('w_lT', [P, P], bf16, kind='ExternalInput')
    w_rT = nc.dram_tensor('w_rT', [P, P], bf16, kind='ExternalInput')
    w_eT = nc.dram_tensor('w_eT', [EDGE_DIM, P], bf16, kind='ExternalInput')
    att_exp = nc.dram_tensor('att_exp', [P, HEADS], bf16, kind='ExternalInput')
    iota_row = nc.dram_tensor('iota_row', [P, P], bf16, kind='ExternalInput')
    iota_col = nc.dram_tensor('iota_col', [P, 1], f32, kind='ExternalInput')
    ones_row = nc.dram_tensor('ones_row', [1, P], bf16, kind='ExternalInput')
    id4 = nc.dram_tensor('id4', [HEADS, HEADS], bf16, kind='ExternalInput')
    bias_lr = nc.dram_tensor('bias_lr', [P, 1], f32, kind='ExternalInput')
    aff = None
    if not trivial_affine:
        # rows: b_l bcast, conv_bias bcast, gamma bcast, beta bcast
        aff = nc.dram_tensor('aff', [P, 4 * P], f32, kind='ExternalInput')

    out_d = nc.dram_tensor('out', [NPC_PAD, P], f32, kind='ExternalOutput')

    from concourse import library_config
    with tile.TileContext(nc) as tc:
        nc.gpsimd.load_library(library_config.mlp)
        with tc.tile_pool(name='const', bufs=1) as cp:
            c_wlT = cp.tile([P, P], bf16)
            nc.sync.dma_start(c_wlT[:], w_lT[:])
            c_wrT = cp.tile([P, P], bf16)
            nc.sync.dma_start(c_wrT[:], w_rT[:])
            c_weT = cp.tile([EDGE_DIM, P], bf16)
            nc.sync.dma_start(c_weT[:], w_eT[:])
            c_att = cp.tile([P, HEADS], bf16)
            nc.sync.dma_start(c_att[:], att_exp[:])
            c_iota = cp.tile([P, P], bf16)
            nc.sync.dma_start(c_iota[:], iota_row[:])
            c_iotac = cp.tile([P, 1], f32)
            nc.sync.dma_start(c_iotac[:], iota_col[:])
            c_ones = cp.tile([1, P], bf16)
            nc.sync.dma_start(c_ones[:], ones_row[:])
            c_id4 = cp.tile([HEADS, HEADS], bf16)
            nc.sync.dma_start(c_id4[:], id4[:])
            c_blr = cp.tile([P, 1], f32)
            nc.sync.dma_start(c_blr[:], bias_lr[:])
            c_xownT = cp.tile([P, NPC_PAD], bf16)
            nc.sync.dma_start(c_xownT[:], x_ownT[:])
            c_aff = None
            if aff is not None:
                c_aff = cp.tile([P, 4 * P], f32)
                nc.sync.dma_start(c_aff[:], aff[:])

            with tc.tile_pool(name='persist', bufs=1) as pp:
                xr_sb = pp.tile([P, NT * P], bf16)      # xr per node tile
                ubuf = pp.tile([P, NT * 132], f32)     # unnorm(128)+denom(4)
                hbuf = pp.tile([P, NT * P], f32)       # post-residual h
                stats = pp.tile([P, NT * 2], f32)      # mean, var interleaved

                # ---------- phase 1: xr for own nodes ----------
                with tc.tile_pool(name='p1psum', bufs=2, space='PSUM') as p1p:
                    for t in range(NT):
                        ps = p1p.tile([P, P], f32)
                        nc.tensor.matmul(ps[:], lhsT=c_xownT[:, t * P:(t + 1) * P],
                                         rhs=c_wrT[:], start=True, stop=True)
                        nc.scalar.copy(xr_sb[:, t * P:(t + 1) * P], ps[:])

                # ---------- phase 2: edge pipeline ----------
                with tc.tile_pool(name='eload', bufs=3) as lp, \
                     tc.tile_pool(name='ework', bufs=3) as wp, \
                     tc.tile_pool(name='psA', bufs=2, space='PSUM') as psA, \
                     tc.tile_pool(name='psC', bufs=2, space='PSUM') as psC, \
                     tc.tile_pool(name='psO', bufs=2, space='PSUM') as psO:
                    chunk_base = 0
                    for t in range(NT):
                        Ct = C_list[t]
                        dl_t = lp.tile([P, Ct], f32, tag='dl')
                        nc.sync.dma_start(
                            dl_t[:], dstloc[:, chunk_base:chunk_base + Ct])
                        te0 = chunk_base * P
                        TW = Ct * P
                        xsT_t = lp.tile([P, CMAX * P], bf16, tag='xsT')
                        nc.sync.dma_start(xsT_t[:, :TW], x_srcT[:, te0:te0 + TW])
                        atr_t = lp.tile([EDGE_DIM, CMAX * P], bf16, tag='atr')
                        nc.sync.dma_start(atr_t[:, :TW], attrT[:, te0:te0 + TW])
                        dr_t = lp.tile([1, CMAX * P], bf16, tag='dr')
                        nc.sync.dma_start(dr_t[:, :TW], dstrow[:, te0:te0 + TW])
                        ps_out = psO.tile([P, 132], f32, tag='out')
                        xr_t = xr_sb[:, t * P:(t + 1) * P]
                        b_sb = wp.tile([P, CMAX * P], bf16, tag='bsb')
                        nc.gpsimd.partition_broadcast(b_sb[:, :TW], dr_t[:1, :TW])
                        IT_t = wp.tile([P, CMAX * P], bf16, tag='IT')
                        nc.vector.tensor_scalar(
                            out=IT_t[:, :TW], in0=b_sb[:, :TW],
                            scalar1=c_iotac[:], scalar2=None, op0=OP.is_equal)
                        I_t = wp.tile([P, CMAX, P], bf16, tag='I')
                        iota_v = c_iota[:, None, :].to_broadcast([P, Ct, P])
                        dl_v = dl_t[:, :, None].to_broadcast([P, Ct, P])
                        nc.vector.tensor_tensor(
                            out=I_t[:, :Ct, :], in0=iota_v, in1=dl_v,
                            op=OP.is_equal)
                        n_super = (Ct + SUPER - 1) // SUPER
                        for s in range(n_super):
                            nch = min(SUPER, Ct - s * SUPER)
                            W = nch * P
                            o0 = s * SUPER * P
                            xsT = xsT_t[:, o0:o0 + W]
                            atr = atr_t[:, o0:o0 + W]

                            # s^T = xj^T + ea^T + xr[dst]^T   (feature-major)
                            ps_sT = psA.tile([P, SUPER * P], f32, tag='sT')
                            nc.tensor.matmul(ps_sT[:, :W], lhsT=c_wlT[:],
                                             rhs=xsT[:, :W], start=True, stop=False)
                            nc.tensor.matmul(ps_sT[:, :W], lhsT=c_weT[:],
                                             rhs=atr[:, :W], start=False, stop=False)
                            nc.tensor.matmul(ps_sT[:, :W], lhsT=xr_t,
                                             rhs=IT_t[:, o0:o0 + W],
                                             start=False, stop=True)

                            # m = lrelu(s + (b_l+b_r))  (bias per feature row)
                            m = wp.tile([P, SUPER * P], bf16, tag='m')
                            nc.scalar.activation(m[:, :W], ps_sT[:, :W], AT.Prelu,
                                                 bias=c_blr[:], alpha=NEG_SLOPE)

                            # logits edge-major: [128e, 4] per chunk
                            ps_ex = psC.tile([P, SUPER * HEADS], f32, tag='lgex')
                            for j in range(nch):
                                nc.tensor.matmul(
                                    ps_ex[:, j * HEADS:(j + 1) * HEADS],
                                    lhsT=m[:, j * P:(j + 1) * P],
                                    rhs=c_att[:], start=True, stop=True)
                            ex_sb = wp.tile([P, SUPER * HEADS], bf16, tag='exs')
                            nc.scalar.activation(ex_sb[:, :nch * HEADS],
                                                 ps_ex[:, :nch * HEADS], AT.Exp)

                            # xj edge-major [e, f]
                            ps_xj = psA.tile([P, SUPER * P], f32, tag='xj')
                            for j in range(nch):
                                nc.tensor.matmul(
                                    ps_xj[:, j * P:(j + 1) * P],
                                    lhsT=xsT[:, j * P:(j + 1) * P],
                                    rhs=c_wlT[:], start=True, stop=True)

                            # msg = [xj * ex_bcast | ex]  -> [128, nch, 132]
                            msg = wp.tile([P, SUPER, 132], bf16, tag='msg')
                            xj_v = ps_xj[:, :W].rearrange('p (c f) -> p c f', c=nch)
                            if aff is not None:
                                # general b_l: xj += b_l (broadcast over rows)
                                xj_sb = wp.tile([P, SUPER * P], bf16, tag='xjb')
                                blv = c_aff[:, 0:P][:, None, :].to_broadcast(
                                    [P, nch, P])
                                nc.vector.tensor_tensor(
                                    out=xj_sb[:, :W].rearrange(
                                        'p (c f) -> p c f', c=nch),
                                    in0=xj_v, in1=blv, op=OP.add)
                                xj_v = xj_sb[:, :W].rearrange(
                                    'p (c f) -> p c f', c=nch)
                            ex_v = (ex_sb[:, :nch * HEADS]
                                    .rearrange('p (c h) -> p c h', c=nch)
                                    [:, :, :, None].to_broadcast(
                                        [P, nch, HEADS, HEAD_DIM]))
                            nc.vector.tensor_tensor(
                                out=msg[:, :nch, 0:P].rearrange(
                                    'p c (h d) -> p c h d', h=HEADS),
                                in0=xj_v.rearrange(
                                    'p c (h d) -> p c h d', h=HEADS),
                                in1=ex_v, op=OP.mult)
                            nc.scalar.copy(
                                msg[:, :nch, P:P + HEADS],
                                ex_sb[:, :nch * HEADS].rearrange(
                                    'p (c h) -> p c h', c=nch))

                            # indicator I[e, c, n] = (dst_local == n)
                            # scatter: ps_out[n, :] += I^T @ msg
                            for j in range(nch):
                                first = (s == 0 and j == 0)
                                last = (s == n_super - 1 and j == nch - 1)
                                nc.tensor.matmul(ps_out[:],
                                                 lhsT=I_t[:, s * SUPER + j, :],
                                                 rhs=msg[:, j, :],
                                                 start=first, stop=last)
                        nc.scalar.copy(
                            ubuf[:, t * 132:(t + 1) * 132], ps_out[:])
                        chunk_base += Ct

                # ---------- phase 3: normalize + silu + residual + LN ----------
                with tc.tile_pool(name='tail', bufs=3) as tp:
                    for t in range(NT):
                        u_sl = ubuf[:, t * 132:t * 132 + P]
                        d_sl = ubuf[:, t * 132 + P:t * 132 + P + HEADS]
                        rv = tp.tile([P, HEADS], f32, tag='rv')
                        nc.vector.tensor_scalar(
                            out=rv[:], in0=d_sl, scalar1=1e-16, scalar2=None,
                            op0=OP.add)
                        rvi = tp.tile([P, HEADS], f32, tag='rvi')
                        nc.vector.reciprocal(rvi[:], rv[:])
                        u = tp.tile([P, P], f32, tag='u')
                        rvi_v = rvi[:, :, None].to_broadcast(
                            [P, HEADS, HEAD_DIM])
                        nc.vector.tensor_tensor(
                            out=u[:].rearrange('p (h d) -> p h d', h=HEADS),
                            in0=u_sl.rearrange('p (h d) -> p h d', h=HEADS),
                            in1=rvi_v, op=OP.mult)
                        if aff is not None:
                            nc.vector.tensor_tensor(
                                out=u[:], in0=u[:], in1=c_aff[:, P:2 * P],
                                op=OP.add)
                        ss = tp.tile([P, P], f32, tag='ss')
                        nc.scalar.activation(ss[:], u[:], AT.Silu)
                        xo = tp.tile([P, P], f32, tag='xo')
                        nc.scalar.dma_start(xo[:], x_own[t * P:(t + 1) * P, :])
                        h_sl = hbuf[:, t * P:(t + 1) * P]
                        nc.vector.tensor_tensor(out=h_sl, in0=ss[:], in1=xo[:],
                                                op=OP.add)
                        bs = tp.tile([P, 6], f32, tag='bs')
                        nc.vector.bn_stats(bs[:], h_sl)
                        nc.vector.bn_aggr(stats[:, t * 2:t * 2 + 2], bs[:])

                    veps = tp.tile([P, NT], f32, tag='veps')
                    var_v = stats[:].rearrange('p (t k) -> p t k', k=2)[:, :, 1]
                    nc.vector.tensor_scalar(out=veps[:], in0=var_v,
                                            scalar1=LN_EPS, scalar2=None,
                                            op0=OP.add)
                    vinv = tp.tile([P, NT], f32, tag='vinv')
                    nc.vector.reciprocal(vinv[:], veps[:])
                    rstd = tp.tile([P, NT], f32, tag='rstd')
                    nc.scalar.activation(rstd[:], vinv[:], AT.Sqrt)

                    for t in range(NT):
                        o = tp.tile([P, P], f32, tag='o')
                        nc.vector.tensor_scalar(
                            out=o[:], in0=hbuf[:, t * P:(t + 1) * P],
                            scalar1=stats[:, t * 2:t * 2 + 1],
                            scalar2=rstd[:, t:t + 1],
                            op0=OP.subtract, op1=OP.mult)
                        if aff is not None:
                            nc.vector.tensor_tensor(
                                out=o[:], in0=o[:], in1=c_aff[:, 2 * P:3 * P],
                                op=OP.mult)
                            nc.vector.tensor_tensor(
                                out=o[:], in0=o[:], in1=c_aff[:, 3 * P:4 * P],
                                op=OP.add)
                        nc.scalar.dma_start(out_d[t * P:(t + 1) * P, :], o[:])

    nc.compile()
    return nc


def kernel(x, edge_index, edge_attr, w_l, b_l, w_r, b_r, w_e, att,
           conv_bias, ln_gamma, ln_beta):
    from concourse.bass_utils import run_bass_kernel_spmd

    x = np.asarray(x, dtype=np.float32)
    edge_index = np.asarray(edge_index)
    edge_attr = np.asarray(edge_attr, dtype=np.float32)
    w_l = np.asarray(w_l, dtype=np.float32)
    b_l = np.asarray(b_l, dtype=np.float32)
    w_r = np.asarray(w_r, dtype=np.float32)
    b_r = np.asarray(b_r, dtype=np.float32)
    w_e = np.asarray(w_e, dtype=np.float32)
    att = np.asarray(att, dtype=np.float32)
    conv_bias = np.asarray(conv_bias, dtype=np.float32)
    ln_gamma = np.asarray(ln_gamma, dtype=np.float32)
    ln_beta = np.asarray(ln_beta, dtype=np.float32)

    N = x.shape[0]
    E = edge_index.shape[1]
    NPC = (N + N_CORES - 1) // N_CORES          # 6250
    NT = (NPC + P - 1) // P                     # 49
    NPC_PAD = NT * P                            # 6272

    src = edge_index[0].astype(np.int64)
    dst = edge_index[1].astype(np.int64)
    core = np.minimum(dst // NPC, N_CORES - 1)

    trivial_affine = (not b_l.any()) and (not conv_bias.any()) and \
        np.all(ln_gamma == 1.0) and (not ln_beta.any())

    # per (core, tile) edge lists, sorted by dst
    order = np.lexsort((dst,))
    src_s, dst_s, core_s = src[order], dst[order], core[order]
    attr_s = edge_attr[order]
    tile_of = (dst_s - core_s * NPC) // P

    counts = np.zeros((N_CORES, NT), dtype=np.int64)
    np.add.at(counts, (core_s, tile_of), 1)
    C_list = [int(max(1, np.max((counts[:, t] + P - 1) // P)))
              for t in range(NT)]
    TOTAL_CHUNKS = sum(C_list)
    EW = TOTAL_CHUNKS * P

    key = (tuple(C_list), trivial_affine)
    if key in _CACHE:
        nc = _CACHE[key]
    else:
        nc = _build_program(C_list, trivial_affine)
        _CACHE[key] = nc

    # chunk start offsets per tile
    tile_chunk0 = np.zeros(NT, dtype=np.int64)
    acc = 0
    for t in range(NT):
        tile_chunk0[t] = acc
        acc += C_list[t]

    # consts shared by all cores
    w_lT_h = np.ascontiguousarray(w_l.T).astype(BF16)
    w_rT_h = np.ascontiguousarray(w_r.T).astype(BF16)
    w_eT_h = np.ascontiguousarray(w_e.T).astype(BF16)
    att_exp_h = np.zeros((P, HEADS), dtype=BF16)
    for h in range(HEADS):
        att_exp_h[h * HEAD_DIM:(h + 1) * HEAD_DIM, h] = att[h]
    iota_row_h = np.broadcast_to(
        np.arange(P, dtype=np.float32), (P, P)).astype(BF16)
    iota_col_h = np.arange(P, dtype=np.float32)[:, None].copy()
    ones_row_h = np.ones((1, P), dtype=BF16)
    id4_h = np.eye(HEADS, dtype=BF16)
    bias_lr_h = (b_l + b_r)[:, None].astype(np.float32).copy()
    aff_h = None
    if not trivial_affine:
        aff_h = np.concatenate([
            np.broadcast_to(b_l, (P, P)),
            np.broadcast_to(conv_bias, (P, P)),
            np.broadcast_to(ln_gamma, (P, P)),
            np.broadcast_to(ln_beta, (P, P))], axis=1).astype(np.float32).copy()

    in_maps = []
    for k in range(N_CORES):
        sel = core_s == k
        ksrc, kdst, ktile = src_s[sel], dst_s[sel], tile_of[sel]
        kattr = attr_s[sel]
        # position of each edge in the padded layout
        # edges already sorted by dst -> grouped by tile, in order
        pos = np.empty(len(ksrc), dtype=np.int64)
        csum = 0
        x_srcT_h = np.zeros((P, EW), dtype=BF16)
        attrT_h = np.zeros((EDGE_DIM, EW), dtype=BF16)
        dstrow_h = np.full((1, EW), -1.0, dtype=BF16)
        dstloc_h = np.full((P, TOTAL_CHUNKS), -1.0, dtype=np.float32)
        for t in range(NT):
            tsel = ktile == t
            n_t = int(tsel.sum())
            base = tile_chunk0[t] * P
            pos[tsel] = base + np.arange(n_t)
            csum += n_t
        x_srcT_h[:, pos] = x[ksrc].T.astype(BF16)
        attrT_h[:, pos] = kattr.T.astype(BF16)
        dloc = (kdst - k * NPC - ktile * P).astype(np.float32)
        dstrow_h[0, pos] = dloc.astype(BF16)
        dstloc_h[pos % P, pos // P] = dloc

        xk = np.zeros((NPC_PAD, P), dtype=np.float32)
        n_own = min(NPC, N - k * NPC)
        xk[:n_own] = x[k * NPC:k * NPC + n_own]
        im = {
            'x_srcT': x_srcT_h, 'attrT': attrT_h, 'dstrow': dstrow_h,
            'dstloc': dstloc_h,
            'x_ownT': np.ascontiguousarray(xk.T).astype(BF16), 'x_own': xk,
            'w_lT': w_lT_h, 'w_rT': w_rT_h, 'w_eT': w_eT_h,
            'att_exp': att_exp_h, 'iota_row': iota_row_h,
            'iota_col': iota_col_h, 'ones_row': ones_row_h, 'id4': id4_h,
            'bias_lr': bias_lr_h,
        }
        if aff_h is not None:
            im['aff'] = aff_h
        in_maps.append(im)

    res = run_bass_kernel_spmd(nc, in_maps, list(range(N_CORES)))
    outs = []
    for k in range(N_CORES):
        n_own = min(NPC, N - k * NPC)
        outs.append(res.results[k]['out'][:n_own])
    return np.concatenate(outs, axis=0)



# revision 3
# speedup vs baseline: 1.2654x; 1.2654x over previous
"""GATv2 block kernel for 8 Trainium2 NeuronCores (Bass/Tile).

Strategy (graph/data parallel over destination nodes):
  - Host sorts edges by destination, shards destination nodes across the
    8 cores (6250 nodes each, padded to 6272 = 49 tiles of 128).
  - Per destination-node tile, edges are padded to multiples of 128
    ("chunks"); chunk counts per tile are maxed across cores so one SPMD
    program serves all 8 cores.
  - Host supplies x[src] pre-gathered AND transposed (x_srcT) so the
    device computes per-edge xl[src] = w_l @ x_src via matmuls with a
    constant stationary operand (no indirect DMA gathers).
  - Segment softmax + scatter-add are matmuls against indicator matrices
    I[e,n] = (dst_local[e] == n).  Both indicator layouts (node-major IT
    for the xr[dst] gather-add, edge-major I_t for the scatter) are
    built on the host and shipped as exact fp8 (values 0/1) to keep the
    GpSimd and Vector engines out of the critical path.
  - The whole edge phase uses one ACT table set (Prelu/Exp/Tanh/Copy);
    SiLU is computed as u*(1+tanh(u/2))/2 so the node tail interleaves
    with the edge pipeline without table switches.  The only switch is a
    single Sqrt at the very end for the LayerNorm rstd.
"""

import numpy as np
import ml_dtypes

BF16 = ml_dtypes.bfloat16
FP8 = ml_dtypes.float8_e4m3fn

P = 128
HEADS = 4
HEAD_DIM = 32
OUT_DIM = 128
IN_DIM = 128
EDGE_DIM = 10
NEG_SLOPE = 0.2
LN_EPS = 1e-5
N_CORES = 8
SUPER = 4   # chunks per superchunk (free dim 512)
GRP = 7     # node tiles per tail group (49 = 7*7)

_CACHE = {}

_PATCHED = []


def _enable_ldw_opt():
    # walrus LDWEIGHTS double-buffering: lets weight loads overlap in-flight
    # matmuls instead of serializing every LDW+MM pair.
    if _PATCHED:
        return
    from concourse import bass_utils as bu
    orig = bu.run_command

    def run_command(argv, **kwargs):
        argv = ['--enable-ldw-opt=true' if a == '--enable-ldw-opt=false' else a
                for a in argv]
        return orig(argv, **kwargs)

    bu.run_command = run_command
    _PATCHED.append(True)


def _build_program(C_list, trivial_affine):
    import concourse.bacc as bacc
    import concourse.tile as tile
    from concourse import mybir

    f32 = mybir.dt.float32
    bf16 = mybir.dt.bfloat16
    fp8 = mybir.dt.float8e4
    AT = mybir.ActivationFunctionType
    OP = mybir.AluOpType

    NT = len(C_list)                       # 49 node tiles per core
    CMAX = max(C_list)
    TOTAL_CHUNKS = sum(C_list)
    NPC_PAD = NT * P                       # 6272
    EW = TOTAL_CHUNKS * P                  # padded edges per core
    NG = (NT + GRP - 1) // GRP             # tail groups

    nc = bacc.Bacc('TRN2', target_bir_lowering=False, debug=False,
                   enable_asserts=True, num_devices=N_CORES)

    # ---- external inputs ----
    x_srcT = nc.dram_tensor('x_srcT', [P, EW], bf16, kind='ExternalInput')
    attrT = nc.dram_tensor('attrT', [EDGE_DIM, EW], bf16, kind='ExternalInput')
    IT_d = nc.dram_tensor('IT_d', [P, EW], fp8, kind='ExternalInput')
    It_d = nc.dram_tensor('It_d', [P, EW], fp8, kind='ExternalInput')
    x_ownT = nc.dram_tensor('x_ownT', [P, NPC_PAD], bf16, kind='ExternalInput')
    x_own = nc.dram_tensor('x_own', [P, NPC_PAD], f32, kind='ExternalInput')
    w_lT = nc.dram_tensor('w_lT', [P, P], bf16, kind='ExternalInput')
    w_rT = nc.dram_tensor('w_rT', [P, P], bf16, kind='ExternalInput')
    w_eT = nc.dram_tensor('w_eT', [EDGE_DIM, P], bf16, kind='ExternalInput')
    att_exp = nc.dram_tensor('att_exp', [P, HEADS], bf16, kind='ExternalInput')
    bias_lr = nc.dram_tensor('bias_lr', [P, 1], f32, kind='ExternalInput')
    aff = None
    if not trivial_affine:
        # rows: b_l bcast, conv_bias bcast, gamma bcast, beta bcast
        aff = nc.dram_tensor('aff', [P, 4 * P], f32, kind='ExternalInput')

    out_d = nc.dram_tensor('out', [P, NPC_PAD], f32, kind='ExternalOutput')

    # chunk start offsets per tile
    tile_chunk0 = []
    acc = 0
    for t in range(NT):
        tile_chunk0.append(acc)
        acc += C_list[t]

    with tile.TileContext(nc) as tc:
        with tc.tile_pool(name='const', bufs=1) as cp:
            c_wlT = cp.tile([P, P], bf16)
            nc.sync.dma_start(c_wlT[:], w_lT[:])
            c_wrT = cp.tile([P, P], bf16)
            nc.sync.dma_start(c_wrT[:], w_rT[:])
            c_weT = cp.tile([EDGE_DIM, P], bf16)
            nc.sync.dma_start(c_weT[:], w_eT[:])
            c_att = cp.tile([P, HEADS], bf16)
            nc.sync.dma_start(c_att[:], att_exp[:])
            c_blr = cp.tile([P, 1], f32)
            nc.sync.dma_start(c_blr[:], bias_lr[:])
            c_xownT = cp.tile([P, NPC_PAD], bf16)
            nc.sync.dma_start(c_xownT[:], x_ownT[:])
            c_aff = None
            if aff is not None:
                c_aff = cp.tile([P, 4 * P], f32)
                nc.sync.dma_start(c_aff[:], aff[:])

            with tc.tile_pool(name='persist', bufs=1) as pp:
                xr_sb = pp.tile([P, NT * P], bf16)      # xr per node tile
                hbuf = pp.tile([P, NT * P], f32)        # post-residual h
                stats = pp.tile([P, NT * 2], f32)       # mean, var interleaved
                vinvb = pp.tile([P, NT], f32)           # 1/(var+eps)

                # ---------- phase 1: xr for own nodes ----------
                with tc.tile_pool(name='p1psum', bufs=2, space='PSUM') as p1p:
                    for t in range(NT):
                        ps = p1p.tile([P, P], f32)
                        nc.tensor.matmul(ps[:], lhsT=c_xownT[:, t * P:(t + 1) * P],
                                         rhs=c_wrT[:], start=True, stop=True)
                        nc.scalar.copy(xr_sb[:, t * P:(t + 1) * P], ps[:])

                # ---------- phase 2: edge pipeline + interleaved tail ----------
                with tc.tile_pool(name='eload', bufs=3) as lp, \
                     tc.tile_pool(name='ework', bufs=3) as wp, \
                     tc.tile_pool(name='ubufp', bufs=2) as up, \
                     tc.tile_pool(name='tailp', bufs=2) as tp, \
                     tc.tile_pool(name='psA', bufs=2, space='PSUM') as psA, \
                     tc.tile_pool(name='psC', bufs=2, space='PSUM') as psC, \
                     tc.tile_pool(name='psO', bufs=2, space='PSUM') as psO:

                    def tail_group(g):
                        g0 = g * GRP
                        gn = min(GRP, NT - g0)
                        GW = gn * P
                        ub = ubuf_g[:, :gn * 132].rearrange(
                            'p (t w) -> p t w', w=132)
                        # alpha denominators -> reciprocal
                        rv = tp.tile([P, GRP * HEADS], f32, tag='rv')
                        nc.vector.tensor_scalar(
                            out=rv[:, :gn * HEADS].rearrange(
                                'p (t h) -> p t h', h=HEADS),
                            in0=ub[:, :, P:P + HEADS],
                            scalar1=1e-16, scalar2=None, op0=OP.add)
                        rvi = tp.tile([P, GRP * HEADS], f32, tag='rvi')
                        nc.vector.reciprocal(rvi[:, :gn * HEADS],
                                             rv[:, :gn * HEADS])
                        # u = unnorm * 1/denom (per head)
                        u = tp.tile([P, GRP * P], f32, tag='u')
                        rvi_v = (rvi[:, :gn * HEADS]
                                 .rearrange('p (t h) -> p t h', h=HEADS)
                                 [:, :, :, None]
                                 .to_broadcast([P, gn, HEADS, HEAD_DIM]))
                        nc.vector.tensor_tensor(
                            out=u[:, :GW].rearrange(
                                'p (t h d) -> p t h d', h=HEADS, d=HEAD_DIM),
                            in0=ub[:, :, :P].rearrange(
                                'p t (h d) -> p t h d', h=HEADS),
                            in1=rvi_v, op=OP.mult)
                        if c_aff is not None:
                            bcv = c_aff[:, P:2 * P][:, None, :].to_broadcast(
                                [P, gn, P])
                            nc.vector.tensor_tensor(
                                out=u[:, :GW].rearrange(
                                    'p (t f) -> p t f', f=P),
                                in0=u[:, :GW].rearrange(
                                    'p (t f) -> p t f', f=P),
                                in1=bcv, op=OP.add)
                        # silu(u) = u * (1 + tanh(u/2)) / 2  (tanh shares the
                        # exp table set -> no ACT table switch mid-pipeline)
                        th = tp.tile([P, GRP * P], f32, tag='th')
                        nc.scalar.activation(th[:, :GW], u[:, :GW], AT.Tanh,
                                             scale=0.5)
                        ss2 = tp.tile([P, GRP * P], f32, tag='ss2')
                        nc.vector.scalar_tensor_tensor(
                            out=ss2[:, :GW], in0=th[:, :GW], scalar=1.0,
                            in1=u[:, :GW], op0=OP.add, op1=OP.mult)
                        xo = tp.tile([P, GRP * P], f32, tag='xo')
                        nc.scalar.dma_start(
                            xo[:, :GW], x_own[:, g0 * P:g0 * P + GW])
                        h_sl = hbuf[:, g0 * P:g0 * P + GW]
                        nc.vector.scalar_tensor_tensor(
                            out=h_sl, in0=ss2[:, :GW], scalar=0.5,
                            in1=xo[:, :GW], op0=OP.mult, op1=OP.add)
                        for i in range(gn):
                            t = g0 + i
                            bs = tp.tile([P, 6], f32, tag='bs')
                            nc.vector.bn_stats(
                                bs[:], hbuf[:, t * P:(t + 1) * P])
                            nc.vector.bn_aggr(stats[:, t * 2:t * 2 + 2], bs[:])
                        veps = tp.tile([P, GRP], f32, tag='veps')
                        var_v = (stats[:, g0 * 2:(g0 + gn) * 2]
                                 .rearrange('p (t k) -> p t k', k=2)[:, :, 1])
                        nc.vector.tensor_scalar(
                            out=veps[:, :gn], in0=var_v, scalar1=LN_EPS,
                            scalar2=None, op0=OP.add)
                        nc.vector.reciprocal(vinvb[:, g0:g0 + gn],
                                             veps[:, :gn])

                    chunk_base = 0
                    ubuf_g = None
                    for t in range(NT):
                        if t % GRP == 0:
                            ubuf_g = up.tile([P, GRP * 132], f32, tag='ubuf')
                        Ct = C_list[t]
                        te0 = chunk_base * P
                        TW = Ct * P
                        xsT_t = lp.tile([P, CMAX * P], bf16, tag='xsT')
                        nc.sync.dma_start(xsT_t[:, :TW], x_srcT[:, te0:te0 + TW])
                        atr_t = lp.tile([EDGE_DIM, CMAX * P], bf16, tag='atr')
                        nc.sync.dma_start(atr_t[:, :TW], attrT[:, te0:te0 + TW])
                        IT_t = lp.tile([P, CMAX * P], fp8, tag='IT')
                        nc.gpsimd.dma_start(IT_t[:, :TW], IT_d[:, te0:te0 + TW])
                        It_t = lp.tile([P, CMAX * P], fp8, tag='It')
                        nc.gpsimd.dma_start(It_t[:, :TW], It_d[:, te0:te0 + TW])
                        ps_out = psO.tile([P, 132], f32, tag='out')
                        xr_t = xr_sb[:, t * P:(t + 1) * P]
                        n_super = (Ct + SUPER - 1) // SUPER
                        for s in range(n_super):
                            nch = min(SUPER, Ct - s * SUPER)
                            W = nch * P
                            o0 = s * SUPER * P
                            xsT = xsT_t[:, o0:o0 + W]
                            atr = atr_t[:, o0:o0 + W]

                            # s^T = xj^T + ea^T + xr[dst]^T   (feature-major)
                            ps_sT = psA.tile([P, SUPER * P], f32, tag='sT')
                            nc.tensor.matmul(ps_sT[:, :W], lhsT=c_wlT[:],
                                             rhs=xsT[:, :W], start=True, stop=False)
                            nc.tensor.matmul(ps_sT[:, :W], lhsT=c_weT[:],
                                             rhs=atr[:, :W], start=False, stop=False)
                            nc.tensor.matmul(ps_sT[:, :W], lhsT=xr_t,
                                             rhs=IT_t[:, o0:o0 + W],
                                             start=False, stop=True)

                            # m = lrelu(s + (b_l+b_r))  (bias per feature row)
                            m = wp.tile([P, SUPER * P], bf16, tag='m')
                            nc.scalar.activation(m[:, :W], ps_sT[:, :W], AT.Prelu,
                                                 bias=c_blr[:], alpha=NEG_SLOPE)

                            # logits edge-major: [128e, 4] per chunk
                            ps_ex = psC.tile([P, SUPER * HEADS], f32, tag='lgex')
                            for j in range(nch):
                                nc.tensor.matmul(
                                    ps_ex[:, j * HEADS:(j + 1) * HEADS],
                                    lhsT=m[:, j * P:(j + 1) * P],
                                    rhs=c_att[:], start=True, stop=True)
                            # msg = [xj * ex_bcast | ex]  -> [128, nch, 132]
                            msg = wp.tile([P, SUPER, 132], bf16, tag='msg')
                            nc.scalar.activation(
                                msg[:, :nch, P:P + HEADS],
                                ps_ex[:, :nch * HEADS].rearrange(
                                    'p (c h) -> p c h', h=HEADS),
                                AT.Exp)

                            # xj edge-major [e, f]
                            ps_xj = psA.tile([P, SUPER * P], f32, tag='xj')
                            for j in range(nch):
                                nc.tensor.matmul(
                                    ps_xj[:, j * P:(j + 1) * P],
                                    lhsT=xsT[:, j * P:(j + 1) * P],
                                    rhs=c_wlT[:], start=True, stop=True)

                            xj_v = ps_xj[:, :W].rearrange('p (c f) -> p c f', c=nch)
                            if c_aff is not None:
                                # general b_l: xj += b_l (broadcast over rows)
                                xj_sb = wp.tile([P, SUPER * P], bf16, tag='xjb')
                                blv = c_aff[:, 0:P][:, None, :].to_broadcast(
                                    [P, nch, P])
                                nc.vector.tensor_tensor(
                                    out=xj_sb[:, :W].rearrange(
                                        'p (c f) -> p c f', c=nch),
                                    in0=xj_v, in1=blv, op=OP.add)
                                xj_v = xj_sb[:, :W].rearrange(
                                    'p (c f) -> p c f', c=nch)
                            ex_v = (msg[:, :nch, P:P + HEADS]
                                    [:, :, :, None].to_broadcast(
                                        [P, nch, HEADS, HEAD_DIM]))
                            nc.vector.tensor_tensor(
                                out=msg[:, :nch, 0:P].rearrange(
                                    'p c (h d) -> p c h d', h=HEADS),
                                in0=xj_v.rearrange(
                                    'p c (h d) -> p c h d', h=HEADS),
                                in1=ex_v, op=OP.mult)

                            # scatter: ps_out[n, :] += I^T @ msg
                            for j in range(nch):
                                first = (s == 0 and j == 0)
                                last = (s == n_super - 1 and j == nch - 1)
                                nc.tensor.matmul(ps_out[:],
                                                 lhsT=It_t[:, o0 + j * P:
                                                           o0 + (j + 1) * P],
                                                 rhs=msg[:, j, :],
                                                 start=first, stop=last)
                        nc.scalar.copy(
                            ubuf_g[:, (t % GRP) * 132:(t % GRP + 1) * 132],
                            ps_out[:])
                        chunk_base += Ct
                        if t % GRP == GRP - 1 or t == NT - 1:
                            tail_group(t // GRP)

                    # ---------- end phase: rstd + normalize + store ----------
                    rstd = tp.tile([P, NT], f32, tag='rstd')
                    nc.scalar.activation(rstd[:], vinvb[:], AT.Sqrt)
                    for g in range(NG):
                        g0 = g * GRP
                        gn = min(GRP, NT - g0)
                        GW = gn * P
                        mu_v = (stats[:, g0 * 2:(g0 + gn) * 2]
                                .rearrange('p (t k) -> p t k', k=2)
                                [:, :, 0:1].to_broadcast([P, gn, P]))
                        o = tp.tile([P, GRP * P], f32, tag='o')
                        nc.vector.tensor_tensor(
                            out=o[:, :GW].rearrange('p (t f) -> p t f', f=P),
                            in0=hbuf[:, g0 * P:g0 * P + GW].rearrange(
                                'p (t f) -> p t f', f=P),
                            in1=mu_v, op=OP.subtract)
                        rs_v = (rstd[:, g0:g0 + gn][:, :, None]
                                .to_broadcast([P, gn, P]))
                        nc.vector.tensor_tensor(
                            out=o[:, :GW].rearrange('p (t f) -> p t f', f=P),
                            in0=o[:, :GW].rearrange('p (t f) -> p t f', f=P),
                            in1=rs_v, op=OP.mult)
                        if c_aff is not None:
                            gv = c_aff[:, 2 * P:3 * P][:, None, :].to_broadcast(
                                [P, gn, P])
                            nc.vector.tensor_tensor(
                                out=o[:, :GW].rearrange('p (t f) -> p t f', f=P),
                                in0=o[:, :GW].rearrange('p (t f) -> p t f', f=P),
                                in1=gv, op=OP.mult)
                            bv = c_aff[:, 3 * P:4 * P][:, None, :].to_broadcast(
                                [P, gn, P])
                            nc.vector.tensor_tensor(
                                out=o[:, :GW].rearrange('p (t f) -> p t f', f=P),
                                in0=o[:, :GW].rearrange('p (t f) -> p t f', f=P),
                                in1=bv, op=OP.add)
                        nc.scalar.dma_start(
                            out_d[:, g0 * P:g0 * P + GW], o[:, :GW])

    nc.compile()
    return nc


def kernel(x, edge_index, edge_attr, w_l, b_l, w_r, b_r, w_e, att,
           conv_bias, ln_gamma, ln_beta):
    from concourse.bass_utils import run_bass_kernel_spmd

    x = np.asarray(x, dtype=np.float32)
    edge_index = np.asarray(edge_index)
    edge_attr = np.asarray(edge_attr, dtype=np.float32)
    w_l = np.asarray(w_l, dtype=np.float32)
    b_l = np.asarray(b_l, dtype=np.float32)
    w_r = np.asarray(w_r, dtype=np.float32)
    b_r = np.asarray(b_r, dtype=np.float32)
    w_e = np.asarray(w_e, dtype=np.float32)
    att = np.asarray(att, dtype=np.float32)
    conv_bias = np.asarray(conv_bias, dtype=np.float32)
    ln_gamma = np.asarray(ln_gamma, dtype=np.float32)
    ln_beta = np.asarray(ln_beta, dtype=np.float32)

    N = x.shape[0]
    NPC = (N + N_CORES - 1) // N_CORES          # 6250
    NT = (NPC + P - 1) // P                     # 49
    NPC_PAD = NT * P                            # 6272

    src = edge_index[0].astype(np.int64)
    dst = edge_index[1].astype(np.int64)
    core = np.minimum(dst // NPC, N_CORES - 1)

    trivial_affine = (not b_l.any()) and (not conv_bias.any()) and \
        np.all(ln_gamma == 1.0) and (not ln_beta.any())

    # per (core, tile) edge lists, sorted by dst
    order = np.lexsort((dst,))
    src_s, dst_s, core_s = src[order], dst[order], core[order]
    attr_s = edge_attr[order]
    tile_of = (dst_s - core_s * NPC) // P

    counts = np.zeros((N_CORES, NT), dtype=np.int64)
    np.add.at(counts, (core_s, tile_of), 1)
    C_list = [int(max(1, np.max((counts[:, t] + P - 1) // P)))
              for t in range(NT)]
    TOTAL_CHUNKS = sum(C_list)
    EW = TOTAL_CHUNKS * P

    key = (tuple(C_list), trivial_affine)
    if key in _CACHE:
        nc = _CACHE[key]
    else:
        nc = _build_program(C_list, trivial_affine)
        _CACHE[key] = nc

    # chunk start offsets per tile
    tile_chunk0 = np.zeros(NT, dtype=np.int64)
    acc = 0
    for t in range(NT):
        tile_chunk0[t] = acc
        acc += C_list[t]

    # consts shared by all cores
    w_lT_h = np.ascontiguousarray(w_l.T).astype(BF16)
    w_rT_h = np.ascontiguousarray(w_r.T).astype(BF16)
    w_eT_h = np.ascontiguousarray(w_e.T).astype(BF16)
    att_exp_h = np.zeros((P, HEADS), dtype=BF16)
    for h in range(HEADS):
        att_exp_h[h * HEAD_DIM:(h + 1) * HEAD_DIM, h] = att[h]
    bias_lr_h = (b_l + b_r)[:, None].astype(np.float32).copy()
    aff_h = None
    if not trivial_affine:
        aff_h = np.concatenate([
            np.broadcast_to(b_l, (P, P)),
            np.broadcast_to(conv_bias, (P, P)),
            np.broadcast_to(ln_gamma, (P, P)),
            np.broadcast_to(ln_beta, (P, P))], axis=1).astype(np.float32).copy()

    in_maps = []
    for k in range(N_CORES):
        sel = core_s == k
        ksrc, kdst, ktile = src_s[sel], dst_s[sel], tile_of[sel]
        kattr = attr_s[sel]
        # position of each edge in the padded layout
        # edges already sorted by dst -> grouped by tile, in order
        pos = np.empty(len(ksrc), dtype=np.int64)
        for t in range(NT):
            tsel = ktile == t
            n_t = int(tsel.sum())
            base = tile_chunk0[t] * P
            pos[tsel] = base + np.arange(n_t)
        x_srcT_h = np.zeros((P, EW), dtype=BF16)
        attrT_h = np.zeros((EDGE_DIM, EW), dtype=BF16)
        x_srcT_h[:, pos] = x[ksrc].T.astype(BF16)
        attrT_h[:, pos] = kattr.T.astype(BF16)
        dloc = (kdst - k * NPC - ktile * P).astype(np.int64)
        # indicator matrices (exact 0/1 in fp8)
        IT_h = np.zeros((P, EW), dtype=FP8)
        IT_h[dloc, pos] = FP8(1.0)
        It_h = np.zeros((P, EW), dtype=FP8)
        It_h[pos % P, (pos // P) * P + dloc] = FP8(1.0)

        xk = np.zeros((NPC_PAD, P), dtype=np.float32)
        n_own = min(NPC, N - k * NPC)
        xk[:n_own] = x[k * NPC:k * NPC + n_own]
        # partition-major own-node features: [p, t*128+f] = xk[t*128+p, f]
        x_own_pm = np.ascontiguousarray(
            xk.reshape(NT, P, P).transpose(1, 0, 2).reshape(P, NT * P))
        im = {
            'x_srcT': x_srcT_h, 'attrT': attrT_h,
            'IT_d': IT_h, 'It_d': It_h,
            'x_ownT': np.ascontiguousarray(xk.T).astype(BF16),
            'x_own': x_own_pm,
            'w_lT': w_lT_h, 'w_rT': w_rT_h, 'w_eT': w_eT_h,
            'att_exp': att_exp_h, 'bias_lr': bias_lr_h,
        }
        if aff_h is not None:
            im['aff'] = aff_h
        in_maps.append(im)

    res = run_bass_kernel_spmd(nc, in_maps, list(range(N_CORES)))
    outs = []
    for k in range(N_CORES):
        n_own = min(NPC, N - k * NPC)
        o = res.results[k]['out']            # [P, NT*P] partition-major
        o = o.reshape(P, NT, P).transpose(1, 0, 2).reshape(NPC_PAD, P)
        outs.append(o[:n_own])
    return np.concatenate(outs, axis=0)


# revision 11
# speedup vs baseline: 1.7263x; 1.3642x over previous
"""GATv2 block kernel for 8 Trainium2 NeuronCores (Bass/Tile).

Strategy (graph/data parallel over destination nodes):
  - Host sorts edges by destination, shards destination nodes across the
    8 cores, splits each core's nodes into tiles of 118 (padded into a
    128-partition frame: rows 118..127 carry the 10 edge-attr channels).
  - Per destination-node tile, edges are padded to multiples of 128
    ("chunks"); chunk counts per tile are maxed across cores so one SPMD
    program serves all 8 cores.
  - Host supplies x[src] pre-gathered AND transposed (x_srcT) so the
    device computes per-edge xl[src] = w_l @ x_src via matmuls with a
    constant stationary operand (no indirect DMA gathers).
  - Segment softmax + scatter-add are matmuls against indicator matrices
    I[e,n] = (dst_local[e] == n), built on the host in exact fp8.
    The node-major indicator is fused with attrT ([118 indicator rows |
    10 attr rows]) and the matching stationary is [xr (118 rows) | w_eT
    (10 rows)], so ONE psum-stream adds both xr[dst] and w_e@attr —
    eliminating a full 800k-column stream from the tensor engine.
  - The whole edge phase uses one ACT table set (Prelu/Exp/Tanh/Copy);
    SiLU is computed as u*(1+tanh(u/2))/2 so the node tail interleaves
    with the edge pipeline without table switches.  Sqrt for the
    LayerNorm rstd runs twice (once mid-kernel for the first 6 tile
    groups, once at the end), costing two table round-trips off the
    critical path.
"""

import numpy as np
import ml_dtypes

BF16 = ml_dtypes.bfloat16
FP8 = ml_dtypes.float8_e4m3fn

P = 128
NPT = 118   # destination nodes per tile (rows 118..127 carry attr)
HEADS = 4
HEAD_DIM = 32
OUT_DIM = 128
IN_DIM = 128
EDGE_DIM = 10
NEG_SLOPE = 0.2
LN_EPS = 1e-5
N_CORES = 8
SUPER = 4   # chunks per superchunk (free dim 512)
GRP = 7     # node tiles per tail group
PF = 3      # edge-load prefetch depth (tiles)

_CACHE = {}


def _build_program(C_list, trivial_affine):
    import concourse.bacc as bacc
    import concourse.tile as tile
    from concourse import mybir

    f32 = mybir.dt.float32
    bf16 = mybir.dt.bfloat16
    fp8 = mybir.dt.float8e4
    AT = mybir.ActivationFunctionType
    OP = mybir.AluOpType

    NT = len(C_list)                       # 53 node tiles per core
    CMAX = max(C_list)
    TOTAL_CHUNKS = sum(C_list)
    NN = NT * NPT                          # 6254 packed own nodes
    NP_PAD = NT * P                        # 6784 (tail frames, 118 used)
    EW = TOTAL_CHUNKS * P                  # padded edges per core
    NG = (NT + GRP - 1) // GRP             # tail groups
    EARLY_G = 6                            # groups normalized mid-kernel
    EARLY_T = 43                           # emit early normalize after this tile

    nc = bacc.Bacc('TRN2', target_bir_lowering=False, debug=False,
                   enable_asserts=True, num_devices=N_CORES)

    # ---- external inputs ----
    x_srcT = nc.dram_tensor('x_srcT', [P, EW], bf16, kind='ExternalInput')
    ITA_d = nc.dram_tensor('ITA_d', [P, EW], fp8, kind='ExternalInput')
    It_d = nc.dram_tensor('It_d', [P, EW], fp8, kind='ExternalInput')
    x_ownT = nc.dram_tensor('x_ownT', [P, NN], bf16, kind='ExternalInput')
    x_own = nc.dram_tensor('x_own', [P, NP_PAD], f32, kind='ExternalInput')
    w_lT = nc.dram_tensor('w_lT', [P, P], bf16, kind='ExternalInput')
    w_rT = nc.dram_tensor('w_rT', [P, P], bf16, kind='ExternalInput')
    w_eT_rep = nc.dram_tensor('w_eT_rep', [EDGE_DIM, NT * P], bf16,
                              kind='ExternalInput')
    att_exp = nc.dram_tensor('att_exp', [P, HEADS], bf16, kind='ExternalInput')
    bias_lr = nc.dram_tensor('bias_lr', [P, 1], f32, kind='ExternalInput')
    aff = None
    if not trivial_affine:
        # rows: b_l bcast, conv_bias bcast, gamma bcast, beta bcast
        aff = nc.dram_tensor('aff', [P, 4 * P], f32, kind='ExternalInput')

    out_d = nc.dram_tensor('out', [P, NP_PAD], f32, kind='ExternalOutput')

    # chunk start offsets per tile
    tile_chunk0 = []
    acc = 0
    for t in range(NT):
        tile_chunk0.append(acc)
        acc += C_list[t]

    with tile.TileContext(nc) as tc:
        with tc.tile_pool(name='const', bufs=1) as cp:
            # x_ownT arrives in 8 slices on the scalar queue so phase 1
            # starts before the whole tensor lands.
            c_xownT = cp.tile([P, NN], bf16)
            NSL = 8
            sl = (NN + NSL - 1) // NSL
            for i in range(NSL):
                a, b = i * sl, min(NN, (i + 1) * sl)
                nc.scalar.dma_start(c_xownT[:, a:b], x_ownT[:, a:b])
            c_wlT = cp.tile([P, P], bf16)
            nc.sync.dma_start(c_wlT[:], w_lT[:])
            c_wrT = cp.tile([P, P], bf16)
            nc.sync.dma_start(c_wrT[:], w_rT[:])
            c_att = cp.tile([P, HEADS], bf16)
            nc.sync.dma_start(c_att[:], att_exp[:])
            c_blr = cp.tile([P, 1], f32)
            nc.sync.dma_start(c_blr[:], bias_lr[:])
            c_aff = None
            if aff is not None:
                c_aff = cp.tile([P, 4 * P], f32)
                nc.sync.dma_start(c_aff[:], aff[:])

            with tc.tile_pool(name='persist', bufs=1) as pp:
                xrw_sb = pp.tile([P, NT * P], bf16)      # [xr|w_eT] per tile
                # rows 118..127 of every tile's stationary = w_eT (one DMA)
                nc.sync.dma_start(xrw_sb[NPT:P, :], w_eT_rep[:])
                hbuf = pp.tile([P, NT * P], f32)         # post-residual h
                stats = pp.tile([P, NT * 2], f32)        # mean, var interleaved
                vinvb = pp.tile([P, NT], f32)            # 1/(var+eps)

                with tc.tile_pool(name='eload', bufs=PF + 1) as lp, \
                     tc.tile_pool(name='ework', bufs=3) as wp, \
                     tc.tile_pool(name='ubufp', bufs=2) as up, \
                     tc.tile_pool(name='tailp', bufs=2) as tp:

                    loads = {}

                    def emit_loads(t):
                        te0 = tile_chunk0[t] * P
                        TW = C_list[t] * P
                        xsT_t = lp.tile([P, CMAX * P], bf16, tag='xsT')
                        nc.sync.dma_start(xsT_t[:, :TW],
                                          x_srcT[:, te0:te0 + TW])
                        ITA_t = lp.tile([P, CMAX * P], fp8, tag='ITA')
                        nc.gpsimd.dma_start(ITA_t[:, :TW],
                                            ITA_d[:, te0:te0 + TW])
                        It_t = lp.tile([P, CMAX * P], fp8, tag='It')
                        nc.gpsimd.dma_start(It_t[:, :TW],
                                            It_d[:, te0:te0 + TW])
                        loads[t] = (xsT_t, ITA_t, It_t)

                    for t in range(min(PF, NT)):
                        emit_loads(t)

                    # -------- phase 1: [xr | w_eT] stationaries --------
                    with tc.tile_pool(name='p1psum', bufs=2,
                                      space='PSUM') as p1p:
                        for t in range(NT):
                            ps = p1p.tile([P, P], f32)
                            nc.tensor.matmul(
                                ps[:NPT, :],
                                lhsT=c_xownT[:, t * NPT:(t + 1) * NPT],
                                rhs=c_wrT[:], start=True, stop=True)
                            nc.scalar.copy(
                                xrw_sb[:NPT, t * P:(t + 1) * P], ps[:NPT, :])

                    # -------- phase 2: edge pipeline + interleaved tail ----
                    def tail_group(g):
                        g0 = g * GRP
                        gn = min(GRP, NT - g0)
                        GW = gn * P
                        ub = ubuf_g[:, :gn * 132].rearrange(
                            'p (t w) -> p t w', w=132)
                        rv = tp.tile([P, GRP * HEADS], f32, tag='rv')
                        nc.vector.tensor_scalar(
                            out=rv[:, :gn * HEADS].rearrange(
                                'p (t h) -> p t h', h=HEADS),
                            in0=ub[:, :, P:P + HEADS],
                            scalar1=1e-16, scalar2=None, op0=OP.add)
                        rvi = tp.tile([P, GRP * HEADS], f32, tag='rvi')
                        nc.vector.reciprocal(rvi[:, :gn * HEADS],
                                             rv[:, :gn * HEADS])
                        u = tp.tile([P, GRP * P], f32, tag='u')
                        rvi_v = (rvi[:, :gn * HEADS]
                                 .rearrange('p (t h) -> p t h', h=HEADS)
                                 [:, :, :, None]
                                 .to_broadcast([P, gn, HEADS, HEAD_DIM]))
                        nc.vector.tensor_tensor(
                            out=u[:, :GW].rearrange(
                                'p (t h d) -> p t h d', h=HEADS, d=HEAD_DIM),
                            in0=ub[:, :, :P].rearrange(
                                'p t (h d) -> p t h d', h=HEADS),
                            in1=rvi_v, op=OP.mult)
                        if c_aff is not None:
                            bcv = c_aff[:, P:2 * P][:, None, :].to_broadcast(
                                [P, gn, P])
                            nc.vector.tensor_tensor(
                                out=u[:, :GW].rearrange(
                                    'p (t f) -> p t f', f=P),
                                in0=u[:, :GW].rearrange(
                                    'p (t f) -> p t f', f=P),
                                in1=bcv, op=OP.add)
                        # silu(u) = u * (1 + tanh(u/2)) / 2  (tanh shares the
                        # exp table set -> no ACT table switch mid-pipeline)
                        th = tp.tile([P, GRP * P], f32, tag='th')
                        nc.scalar.activation(th[:, :GW], u[:, :GW], AT.Tanh,
                                             scale=0.5)
                        ss2 = tp.tile([P, GRP * P], f32, tag='ss2')
                        nc.vector.scalar_tensor_tensor(
                            out=ss2[:, :GW], in0=th[:, :GW], scalar=1.0,
                            in1=u[:, :GW], op0=OP.add, op1=OP.mult)
                        xo = tp.tile([P, GRP * P], f32, tag='xo')
                        nc.scalar.dma_start(
                            xo[:, :GW], x_own[:, g0 * P:g0 * P + GW])
                        h_sl = hbuf[:, g0 * P:g0 * P + GW]
                        nc.vector.scalar_tensor_tensor(
                            out=h_sl, in0=ss2[:, :GW], scalar=0.5,
                            in1=xo[:, :GW], op0=OP.mult, op1=OP.add)
                        for i in range(gn):
                            tt_ = g0 + i
                            bs = tp.tile([P, 6], f32, tag='bs')
                            nc.vector.bn_stats(
                                bs[:], hbuf[:, tt_ * P:(tt_ + 1) * P])
                            nc.vector.bn_aggr(stats[:, tt_ * 2:tt_ * 2 + 2],
                                              bs[:])
                        veps = tp.tile([P, GRP], f32, tag='veps')
                        var_v = (stats[:, g0 * 2:(g0 + gn) * 2]
                                 .rearrange('p (t k) -> p t k', k=2)[:, :, 1])
                        nc.vector.tensor_scalar(
                            out=veps[:, :gn], in0=var_v, scalar1=LN_EPS,
                            scalar2=None, op0=OP.add)
                        nc.vector.reciprocal(vinvb[:, g0:g0 + gn],
                                             veps[:, :gn])

                    def normalize(g, rstd_ap):
                        g0 = g * GRP
                        gn = min(GRP, NT - g0)
                        GW = gn * P
                        mu_v = (stats[:, g0 * 2:(g0 + gn) * 2]
                                .rearrange('p (t k) -> p t k', k=2)
                                [:, :, 0:1].to_broadcast([P, gn, P]))
                        o = tp.tile([P, GRP * P], f32, tag='o')
                        nc.vector.tensor_tensor(
                            out=o[:, :GW].rearrange('p (t f) -> p t f', f=P),
                            in0=hbuf[:, g0 * P:g0 * P + GW].rearrange(
                                'p (t f) -> p t f', f=P),
                            in1=mu_v, op=OP.subtract)
                        rs_v = rstd_ap[:, :, None].to_broadcast([P, gn, P])
                        nc.vector.tensor_tensor(
                            out=o[:, :GW].rearrange('p (t f) -> p t f', f=P),
                            in0=o[:, :GW].rearrange('p (t f) -> p t f', f=P),
                            in1=rs_v, op=OP.mult)
                        if c_aff is not None:
                            gv = c_aff[:, 2 * P:3 * P][:, None, :]\
                                .to_broadcast([P, gn, P])
                            nc.vector.tensor_tensor(
                                out=o[:, :GW].rearrange(
                                    'p (t f) -> p t f', f=P),
                                in0=o[:, :GW].rearrange(
                                    'p (t f) -> p t f', f=P),
                                in1=gv, op=OP.mult)
                            bv = c_aff[:, 3 * P:4 * P][:, None, :]\
                                .to_broadcast([P, gn, P])
                            nc.vector.tensor_tensor(
                                out=o[:, :GW].rearrange(
                                    'p (t f) -> p t f', f=P),
                                in0=o[:, :GW].rearrange(
                                    'p (t f) -> p t f', f=P),
                                in1=bv, op=OP.add)
                        nc.scalar.dma_start(
                            out_d[:, g0 * P:g0 * P + GW], o[:, :GW])

                    with tc.tile_pool(name='psA', bufs=2, space='PSUM') as psA, \
                         tc.tile_pool(name='psC', bufs=2, space='PSUM') as psC, \
                         tc.tile_pool(name='psO', bufs=2, space='PSUM') as psO:
                        ubuf_g = None
                        for t in range(NT):
                            if t % GRP == 0:
                                ubuf_g = up.tile([P, GRP * 132], f32,
                                                 tag='ubuf')
                            if t + PF < NT:
                                emit_loads(t + PF)
                            xsT_t, ITA_t, It_t = loads.pop(t)
                            Ct = C_list[t]
                            ps_out = psO.tile([P, 132], f32, tag='out')
                            xrw_t = xrw_sb[:, t * P:(t + 1) * P]
                            n_super = (Ct + SUPER - 1) // SUPER
                            for s in range(n_super):
                                nch = min(SUPER, Ct - s * SUPER)
                                W = nch * P
                                o0 = s * SUPER * P
                                xsT = xsT_t[:, o0:o0 + W]

                                # s^T = xj^T + ea^T + xr[dst]^T (feature-major)
                                ps_sT = psA.tile([P, SUPER * P], f32, tag='sT')
                                nc.tensor.matmul(ps_sT[:, :W], lhsT=c_wlT[:],
                                                 rhs=xsT[:, :W],
                                                 start=True, stop=False)
                                nc.tensor.matmul(ps_sT[:, :W], lhsT=xrw_t,
                                                 rhs=ITA_t[:, o0:o0 + W],
                                                 start=False, stop=True)

                                # m = lrelu(s + (b_l+b_r))
                                m = wp.tile([P, SUPER * P], bf16, tag='m')
                                nc.scalar.activation(m[:, :W], ps_sT[:, :W],
                                                     AT.Prelu, bias=c_blr[:],
                                                     alpha=NEG_SLOPE)

                                # logits edge-major: [128e, 4] per chunk
                                ps_ex = psC.tile([P, SUPER * HEADS], f32,
                                                 tag='lgex')
                                for j in range(nch):
                                    nc.tensor.matmul(
                                        ps_ex[:, j * HEADS:(j + 1) * HEADS],
                                        lhsT=m[:, j * P:(j + 1) * P],
                                        rhs=c_att[:], start=True, stop=True)
                                # msg = [xj * ex_bcast | ex] -> [128, nch, 132]
                                msg = wp.tile([P, SUPER, 132], bf16, tag='msg')
                                nc.scalar.activation(
                                    msg[:, :nch, P:P + HEADS],
                                    ps_ex[:, :nch * HEADS].rearrange(
                                        'p (c h) -> p c h', h=HEADS),
                                    AT.Exp)

                                # xj edge-major [e, f]
                                ps_xj = psA.tile([P, SUPER * P], f32, tag='xj')
                                for j in range(nch):
                                    nc.tensor.matmul(
                                        ps_xj[:, j * P:(j + 1) * P],
                                        lhsT=xsT[:, j * P:(j + 1) * P],
                                        rhs=c_wlT[:], start=True, stop=True)

                                xj_v = ps_xj[:, :W].rearrange(
                                    'p (c f) -> p c f', c=nch)
                                if c_aff is not None:
                                    xj_sb = wp.tile([P, SUPER * P], bf16,
                                                    tag='xjb')
                                    blv = c_aff[:, 0:P][:, None, :]\
                                        .to_broadcast([P, nch, P])
                                    nc.vector.tensor_tensor(
                                        out=xj_sb[:, :W].rearrange(
                                            'p (c f) -> p c f', c=nch),
                                        in0=xj_v, in1=blv, op=OP.add)
                                    xj_v = xj_sb[:, :W].rearrange(
                                        'p (c f) -> p c f', c=nch)
                                ex_v = (msg[:, :nch, P:P + HEADS]
                                        [:, :, :, None].to_broadcast(
                                            [P, nch, HEADS, HEAD_DIM]))
                                nc.vector.tensor_tensor(
                                    out=msg[:, :nch, 0:P].rearrange(
                                        'p c (h d) -> p c h d', h=HEADS),
                                    in0=xj_v.rearrange(
                                        'p c (h d) -> p c h d', h=HEADS),
                                    in1=ex_v, op=OP.mult)

                                # scatter: ps_out[n, :] += I^T @ msg
                                for j in range(nch):
                                    first = (s == 0 and j == 0)
                                    last = (s == n_super - 1 and j == nch - 1)
                                    nc.tensor.matmul(
                                        ps_out[:],
                                        lhsT=It_t[:, o0 + j * P:
                                                  o0 + (j + 1) * P],
                                        rhs=msg[:, j, :],
                                        start=first, stop=last)
                            nc.vector.tensor_scalar(
                                out=ubuf_g[:, (t % GRP) * 132:
                                           (t % GRP + 1) * 132],
                                in0=ps_out[:], scalar1=0.0, scalar2=None,
                                op0=OP.add)
                            if t % GRP == GRP - 1 or t == NT - 1:
                                tail_group(t // GRP)
                            if t == EARLY_T:
                                # groups 0..EARLY_G-1 are long done: sqrt +
                                # normalize them now (2 ACT table swaps, off
                                # the critical path)
                                rstdE = tp.tile([P, EARLY_G * GRP], f32,
                                                tag='rstdE')
                                nc.scalar.activation(
                                    rstdE[:], vinvb[:, :EARLY_G * GRP],
                                    AT.Sqrt)
                                for g in range(EARLY_G):
                                    normalize(
                                        g, rstdE[:, g * GRP:(g + 1) * GRP])

                        # ---------- end: remaining groups ----------
                        nrem = NT - EARLY_G * GRP
                        rstdL = tp.tile([P, nrem], f32, tag='rstdL')
                        nc.scalar.activation(
                            rstdL[:], vinvb[:, EARLY_G * GRP:NT], AT.Sqrt)
                        for g in range(EARLY_G, NG):
                            g0 = g * GRP
                            gn = min(GRP, NT - g0)
                            normalize(g, rstdL[:, g0 - EARLY_G * GRP:
                                               g0 - EARLY_G * GRP + gn])

    nc.compile()
    return nc


def kernel(x, edge_index, edge_attr, w_l, b_l, w_r, b_r, w_e, att,
           conv_bias, ln_gamma, ln_beta):
    from concourse.bass_utils import run_bass_kernel_spmd

    x = np.asarray(x, dtype=np.float32)
    edge_index = np.asarray(edge_index)
    edge_attr = np.asarray(edge_attr, dtype=np.float32)
    w_l = np.asarray(w_l, dtype=np.float32)
    b_l = np.asarray(b_l, dtype=np.float32)
    w_r = np.asarray(w_r, dtype=np.float32)
    b_r = np.asarray(b_r, dtype=np.float32)
    w_e = np.asarray(w_e, dtype=np.float32)
    att = np.asarray(att, dtype=np.float32)
    conv_bias = np.asarray(conv_bias, dtype=np.float32)
    ln_gamma = np.asarray(ln_gamma, dtype=np.float32)
    ln_beta = np.asarray(ln_beta, dtype=np.float32)

    N = x.shape[0]
    NPC = (N + N_CORES - 1) // N_CORES          # 6250
    NT = (NPC + NPT - 1) // NPT                 # 53
    NN = NT * NPT                               # 6254
    NP_PAD = NT * P                             # 6784

    src = edge_index[0].astype(np.int64)
    dst = edge_index[1].astype(np.int64)
    core = np.minimum(dst // NPC, N_CORES - 1)

    trivial_affine = (not b_l.any()) and (not conv_bias.any()) and \
        np.all(ln_gamma == 1.0) and (not ln_beta.any())

    # per (core, tile) edge lists, sorted by dst
    order = np.lexsort((dst,))
    src_s, dst_s, core_s = src[order], dst[order], core[order]
    attr_s = edge_attr[order]
    local_s = dst_s - core_s * NPC
    tile_of = local_s // NPT

    counts = np.zeros((N_CORES, NT), dtype=np.int64)
    np.add.at(counts, (core_s, tile_of), 1)
    C_list = [int(max(1, np.max((counts[:, t] + P - 1) // P)))
              for t in range(NT)]
    TOTAL_CHUNKS = sum(C_list)
    EW = TOTAL_CHUNKS * P

    key = (tuple(C_list), trivial_affine)
    if key in _CACHE:
        nc = _CACHE[key]
    else:
        nc = _build_program(C_list, trivial_affine)
        _CACHE[key] = nc

    # chunk start offsets per tile
    tile_chunk0 = np.zeros(NT, dtype=np.int64)
    acc = 0
    for t in range(NT):
        tile_chunk0[t] = acc
        acc += C_list[t]

    # consts shared by all cores
    w_lT_h = np.ascontiguousarray(w_l.T).astype(BF16)
    w_rT_h = np.ascontiguousarray(w_r.T).astype(BF16)
    w_eT_rep_h = np.ascontiguousarray(
        np.tile(w_e.T.astype(BF16), (1, NT)))
    att_exp_h = np.zeros((P, HEADS), dtype=BF16)
    for h in range(HEADS):
        att_exp_h[h * HEAD_DIM:(h + 1) * HEAD_DIM, h] = att[h]
    bias_lr_h = (b_l + b_r)[:, None].astype(np.float32).copy()
    aff_h = None
    if not trivial_affine:
        aff_h = np.concatenate([
            np.broadcast_to(b_l, (P, P)),
            np.broadcast_to(conv_bias, (P, P)),
            np.broadcast_to(ln_gamma, (P, P)),
            np.broadcast_to(ln_beta, (P, P))], axis=1).astype(np.float32).copy()

    in_maps = []
    for k in range(N_CORES):
        sel = core_s == k
        ksrc, klocal, ktile = src_s[sel], local_s[sel], tile_of[sel]
        kattr = attr_s[sel]
        # position of each edge in the padded layout
        # edges already sorted by dst -> grouped by tile, in order
        pos = np.empty(len(ksrc), dtype=np.int64)
        for t in range(NT):
            tsel = ktile == t
            n_t = int(tsel.sum())
            base = tile_chunk0[t] * P
            pos[tsel] = base + np.arange(n_t)
        x_srcT_h = np.zeros((P, EW), dtype=BF16)
        x_srcT_h[:, pos] = x[ksrc].T.astype(BF16)
        dloc = (klocal - ktile * NPT).astype(np.int64)
        # fused node-major indicator (rows 0..117) + attrT (rows 118..127)
        ITA_h = np.zeros((P, EW), dtype=FP8)
        ITA_h[dloc, pos] = FP8(1.0)
        ITA_h[NPT:, :][:, pos] = kattr.T.astype(FP8)
        # edge-major indicator
        It_h = np.zeros((P, EW), dtype=FP8)
        It_h[pos % P, (pos // P) * P + dloc] = FP8(1.0)

        xk = np.zeros((NN, P), dtype=np.float32)
        n_own = min(NPC, N - k * NPC)
        xk[:n_own] = x[k * NPC:k * NPC + n_own]
        # feature-major for phase 1
        x_ownT_h = np.ascontiguousarray(xk.T).astype(BF16)
        # partition-major tail frames: [p, t*128+f] = xk[t*118+p, f], p<118
        x_own_pm = np.zeros((P, NP_PAD), dtype=np.float32)
        x_own_pm.reshape(P, NT, P)[:NPT] = \
            xk.reshape(NT, NPT, P).transpose(1, 0, 2)
        im = {
            'x_srcT': x_srcT_h, 'ITA_d': ITA_h, 'It_d': It_h,
            'x_ownT': x_ownT_h, 'x_own': x_own_pm,
            'w_lT': w_lT_h, 'w_rT': w_rT_h, 'w_eT_rep': w_eT_rep_h,
            'att_exp': att_exp_h, 'bias_lr': bias_lr_h,
        }
        if aff_h is not None:
            im['aff'] = aff_h
        in_maps.append(im)

    res = run_bass_kernel_spmd(nc, in_maps, list(range(N_CORES)))
    outs = []
    for k in range(N_CORES):
        n_own = min(NPC, N - k * NPC)
        o = res.results[k]['out']            # [P, NT*P] partition-major
        o = o.reshape(P, NT, P)[:NPT].transpose(1, 0, 2).reshape(NN, P)
        outs.append(o[:n_own])
    return np.concatenate(outs, axis=0)
